# revision 13
# baseline (speedup 1.0000x reference)
"""Trainium2 Bass kernel for nn_DetectionLoss (histogram_binning).

Computes: ce_mean + coeff * cs_mean over N=16.7M (logit-pair, label) rows,
where coeff is derived from the 2x2 confusion matrix of argmax predictions.

Strategy (data-parallel over 8 NeuronCores, N sharded along axis 0):
  Per element, with d = x1 - x0 and label l in {0,1}:
    ce_i   = softplus(d) - l*d          (== logsumexp CE for 2 classes)
    pred_i = [d > 0]                    (argmax, ties -> class 0)
    cs_i   = l * (1 - pred_i)           (M_COST[pred, l] = [pred=0 & l=1])
  Each core reduces 5 quantities to per-partition partial sums:
    S_spf = sum softplus(d)   (ACT engine, Softplus + accum_out)
    S_ld  = sum l*d           (DVE scalar_tensor_tensor + accum_out)
    TP    = sum l*[d>0]       (DVE scalar_tensor_tensor is_gt*mult + accum_out)
    P1    = sum [d>0]         (DVE tensor_scalar is_gt + accum_out)
    N1    = sum l             (GPSIMD tensor_scalar + accum_out)
  Host combines partials in float64:
    CE_sum = S_spf - S_ld;  FN = N1-TP; FP = P1-TP; TN = N-N1-P1+TP
    sens = TP/max(N1,1); prec = TP/max(P1,1)
    coeff = -0.5*log(max(sens*prec,1e-30)) if all 4 cells nonzero else 1.0
    result = CE_sum/N + coeff * FN/N
"""

import numpy as np

N_TOTAL = 16777216
N_CORES = 8
N_LOC = N_TOTAL // N_CORES  # 2097152
P = 128
F_DMA = 4096  # label elems per partition per DMA tile (outputs tile = 2*F_DMA)
F_C = 2048    # compute sub-tile free size
LAMBD = 1.0


def build_bass_kernel(n_loc=N_LOC, f_dma=F_DMA, f_c=F_C, d_on_gpsimd=True):
    """Build the per-core Bass module. Returns (nc, ncol).

    Engine split (per DMA tile of [128, f_dma] label elems):
      GPSIMD: d = x1 - x0 (one TT over the whole tile)
      ACT:    exp(d) -> g_e; ln(g_e + 1) in-place accum -> sum softplus;
              sign(d) accum -> (2*p1 + zeros - n) [one table set for all]
      DVE:    per f_c sub-tile: l*d (stt, accum), [l*d>0] (TS, accum=TP)
    n1 = sum(labels) is computed host-side during the int64->f32 cast.
    """
    from contextlib import ExitStack

    import concourse.bacc as bacc
    import concourse.tile as tile
    from concourse import mybir

    assert n_loc % (P * f_dma) == 0 and f_dma % f_c == 0
    t_dma = n_loc // (P * f_dma)
    subs = f_dma // f_c
    ncol = t_dma * subs
    f32 = mybir.dt.float32
    Alu = mybir.AluOpType
    Act = mybir.ActivationFunctionType

    nc = bacc.Bacc(None)
    outs = nc.declare_dram_parameter("outputs", [n_loc, 2], f32, isOutput=False)
    labs = nc.declare_dram_parameter("labels", [n_loc], f32, isOutput=False)
    spf_o = nc.declare_dram_parameter("spf_p", [P, t_dma], f32, isOutput=True)
    ld_o = nc.declare_dram_parameter("ld_p", [P, ncol], f32, isOutput=True)
    tp_o = nc.declare_dram_parameter("tp_p", [P, ncol], f32, isOutput=True)
    sgn_o = nc.declare_dram_parameter("sgn_p", [P, t_dma], f32, isOutput=True)

    outs_t = outs.rearrange("(t p f) c -> t p (f c)", p=P, f=f_dma)
    labs_t = labs.rearrange("(t p f) -> t p f", p=P, f=f_dma)

    with ExitStack() as ctx:
        tc = ctx.enter_context(tile.TileContext(nc))
        dma_pool = ctx.enter_context(tc.tile_pool(name="dma", bufs=2))
        dpool = ctx.enter_context(tc.tile_pool(name="d", bufs=2))
        gpool = ctx.enter_context(tc.tile_pool(name="garbage", bufs=1))
        apool = ctx.enter_context(tc.tile_pool(name="accs", bufs=1))

        spf_a = apool.tile([P, t_dma], f32, tag="spf_a")
        ld_a = apool.tile([P, ncol], f32, tag="ld_a")
        tp_a = apool.tile([P, ncol], f32, tag="tp_a")
        sgn_a = apool.tile([P, t_dma], f32, tag="sgn_a")

        # Garbage destinations: accum_out is what we keep; the elementwise
        # outputs are required by the ISA. g_e/g_ld are re-read, but only
        # by the same engine that wrote them.
        g_e = gpool.tile([P, f_dma], f32, tag="g_e")
        g_ld = gpool.tile([P, f_c], f32, tag="g_ld")
        g_tp = gpool.tile([P, f_c], f32, tag="g_tp")

        for t in range(t_dma):
            ot = dma_pool.tile([P, 2 * f_dma], f32, tag="ot")
            lt = dma_pool.tile([P, f_dma], f32, tag="lt")
            nc.sync.dma_start(out=ot, in_=outs_t[t])
            nc.sync.dma_start(out=lt, in_=labs_t[t])
            ot3 = ot.rearrange("p (f c) -> p f c", c=2)
            x0 = ot3[:, :, 0]
            x1 = ot3[:, :, 1]
            dt_ = dpool.tile([P, f_dma], f32, tag="d")
            if d_on_gpsimd:
                nc.gpsimd.tensor_tensor(out=dt_, in0=x1, in1=x0, op=Alu.subtract)
            else:
                nc.vector.tensor_tensor(out=dt_, in0=x1, in1=x0, op=Alu.subtract)
            # softplus(d) = ln(exp(d) + 1); Exp/Ln/Sign all live in the
            # natural_log_exp_and_others ACT table set (single load).
            # |d| <~ 9 here so exp(d) stays well inside fp32 range.
            nc.scalar.activation(out=g_e, in_=dt_, func=Act.Exp)
            nc.scalar.activation(
                out=g_e, in_=g_e, func=Act.Ln, bias=1.0,
                accum_out=spf_a[:, t:t + 1],
            )
            # sum sign(d) = 2*p1 + zeros - n_loc; host corrects (zeros ~ 0,
            # each zero shifts FP/TN by 0.5 -> ~1e-8 on the final scalar).
            nc.scalar.activation(
                out=g_e, in_=dt_, func=Act.Sign,
                accum_out=sgn_a[:, t:t + 1],
            )
            for s in range(subs):
                col = t * subs + s
                sl = slice(s * f_c, (s + 1) * f_c)
                nc.vector.scalar_tensor_tensor(
                    out=g_ld, in0=dt_[:, sl], scalar=0.0, in1=lt[:, sl],
                    op0=Alu.bypass, op1=Alu.mult,
                    accum_out=ld_a[:, col:col + 1],
                )
                nc.vector.tensor_scalar(
                    out=g_tp, in0=g_ld, scalar1=0.0, scalar2=None,
                    op0=Alu.is_gt, op1=Alu.add,
                    accum_out=tp_a[:, col:col + 1],
                )

        nc.sync.dma_start(out=spf_o[:, :], in_=spf_a)
        nc.sync.dma_start(out=ld_o[:, :], in_=ld_a)
        nc.sync.dma_start(out=tp_o[:, :], in_=tp_a)
        nc.sync.dma_start(out=sgn_o[:, :], in_=sgn_a)

    nc.finalize()
    return nc, ncol


def make_in_maps(outputs, labels):
    """Shard full inputs into per-core in_maps (labels cast to f32)."""
    outputs = np.asarray(outputs)
    if outputs.dtype != np.float32:
        outputs = outputs.astype(np.float32)
    labels_f = np.asarray(labels).astype(np.float32)
    in_maps = []
    for c in range(N_CORES):
        sl = slice(c * N_LOC, (c + 1) * N_LOC)
        in_maps.append({"outputs": outputs[sl], "labels": labels_f[sl]})
    return in_maps


def finish_host(per_core_results, n1, n_total=N_TOTAL):
    """Combine per-core partial sums into the final scalar (float64 math).

    n1 = exact sum(labels), computed host-side. p1 is recovered from
    sum(sign(d)) = p1 - (n_total - p1 - zeros).
    """
    s_spf = s_ld = tp = s_sgn = 0.0
    for r in per_core_results:
        s_spf += float(np.sum(r["spf_p"], dtype=np.float64))
        s_ld += float(np.sum(r["ld_p"], dtype=np.float64))
        tp += float(np.sum(r["tp_p"], dtype=np.float64))
        s_sgn += float(np.sum(r["sgn_p"], dtype=np.float64))

    n1 = float(n1)
    p1 = (s_sgn + n_total) / 2.0
    ce_mean = (s_spf - s_ld) / n_total
    fn = n1 - tp
    fp = p1 - tp
    tn = n_total - n1 - p1 + tp
    all_nonzero = (tp != 0.0) and (tn != 0.0) and (fp != 0.0) and (fn != 0.0)
    sens = tp / max(tp + fn, 1.0)
    prec = tp / max(tp + fp, 1.0)
    gm_log = -0.5 * np.log(max(sens * prec, 1e-30))
    coeff = gm_log * LAMBD if all_nonzero else LAMBD
    cs_mean = fn / n_total
    return np.asarray(ce_mean + coeff * cs_mean, dtype=np.float32)


_CACHED = {}


def kernel(outputs, labels):
    from concourse.bass_utils import run_bass_kernel_spmd

    if "nc" not in _CACHED:
        _CACHED["nc"], _ = build_bass_kernel()
    nc = _CACHED["nc"]
    n1 = int(np.asarray(labels).sum())  # exact (labels are 0/1 ints)
    in_maps = make_in_maps(outputs, labels)
    res = run_bass_kernel_spmd(nc, in_maps, core_ids=list(range(N_CORES)))
    return finish_host(res.results, n1)


# revision 23
# speedup vs baseline: 1.1952x; 1.1952x over previous
"""Trainium2 Bass kernel for nn_DetectionLoss (histogram_binning).

Computes: ce_mean + coeff * cs_mean over N=16.7M (logit-pair, label) rows,
where coeff is derived from the 2x2 confusion matrix of argmax predictions.

Strategy (data-parallel over 8 NeuronCores, N sharded along axis 0):
  Per element, with d = x1 - x0 and label l in {0,1}:
    ce_i   = softplus(d) - l*d          (== logsumexp CE for 2 classes)
    pred_i = [d > 0]                    (argmax, ties -> class 0)
    cs_i   = l * (1 - pred_i)           (M_COST[pred, l] = [pred=0 & l=1])
  Each core reduces 5 quantities to per-partition partial sums:
    S_spf = sum softplus(d)   (ACT engine, Softplus + accum_out)
    S_ld  = sum l*d           (DVE scalar_tensor_tensor + accum_out)
    TP    = sum l*[d>0]       (DVE scalar_tensor_tensor is_gt*mult + accum_out)
    P1    = sum [d>0]         (DVE tensor_scalar is_gt + accum_out)
    N1    = sum l             (GPSIMD tensor_scalar + accum_out)
  Host combines partials in float64:
    CE_sum = S_spf - S_ld;  FN = N1-TP; FP = P1-TP; TN = N-N1-P1+TP
    sens = TP/max(N1,1); prec = TP/max(P1,1)
    coeff = -0.5*log(max(sens*prec,1e-30)) if all 4 cells nonzero else 1.0
    result = CE_sum/N + coeff * FN/N
"""

import numpy as np

N_TOTAL = 16777216
N_CORES = 8
N_LOC = N_TOTAL // N_CORES  # 2097152
P = 128
F_DMA = 4096  # label elems per partition per DMA tile (outputs tile = 2*F_DMA)
F_C = 2048    # compute sub-tile free size
LAMBD = 1.0


def _tile_plan(per_part):
    """Tapered DMA-tile sizes (per-partition label elems). Large tiles
    amortize ACT per-op overhead; small tail tiles shorten the post-DMA
    dependency chain (DMA -> d -> exp -> ln)."""
    if per_part == 16384:
        return [4096, 4096, 4096, 2048, 1024, 1024]
    # generic fallback: descending power-of-2 chunks
    plan = []
    rem = per_part
    while rem > 0:
        f = 1 << (rem.bit_length() - 1)
        f = min(f, 4096)
        plan.append(f)
        rem -= f
    return plan


def build_bass_kernel(n_loc=N_LOC, f_c=F_C, d_on_gpsimd=True):
    """Build the per-core Bass module. Returns (nc, ncol).

    Engine split per DMA tile of [128, f] label elems (f from _tile_plan):
      GPSIMD: d = x1 - x0 (one TT over the tile)
      ACT:    exp(d) -> g_e; ln(g_e + 1) in-place, accum -> sum softplus
      DVE:    per <=f_c sub-tile: l*d (stt fp32xbf16, accum); pred=[d>0]
              as bf16 (TS, no accum so the 2x perf mode stays on)
      PE:     p1 += ones^T @ pred (exact count);  TP: diagonal of
              sum_chunks l_chunk^T @ pred_chunk accumulated in PSUM
    n1 = sum(labels) is computed host-side during the int64->bf16 cast.
    Labels travel as bf16 (exact for 0/1) to cut DMA bytes.
    """
    from contextlib import ExitStack

    import concourse.bacc as bacc
    import concourse.tile as tile
    from concourse import mybir

    per_part = n_loc // P
    plan = _tile_plan(per_part)
    assert sum(plan) == per_part
    f32 = mybir.dt.float32
    bf16 = mybir.dt.bfloat16
    Alu = mybir.AluOpType
    Act = mybir.ActivationFunctionType

    subcols = []  # (tile_idx, row_base, sub_off, sub_len)
    row = 0
    for ti, f in enumerate(plan):
        for off in range(0, f, f_c):
            subcols.append((ti, row, off, min(f_c, f - off)))
        row += P * f
    ncol = len(subcols)
    n_tiles = len(plan)

    nc = bacc.Bacc(None)
    outs = nc.declare_dram_parameter("outputs", [n_loc, 2], f32, isOutput=False)
    labs = nc.declare_dram_parameter("labels", [n_loc], bf16, isOutput=False)
    w_p1 = min(512, min(flen for (_, _, _, flen) in subcols))
    spf_o = nc.declare_dram_parameter("spf_p", [P, n_tiles], f32, isOutput=True)
    ld_o = nc.declare_dram_parameter("ld_p", [P, ncol], f32, isOutput=True)
    p1_o = nc.declare_dram_parameter("p1_p", [1, w_p1], f32, isOutput=True)
    tp_o = nc.declare_dram_parameter("tp_p", [P, P], f32, isOutput=True)

    n_mm_p1 = sum(len(range(0, flen, w_p1)) for (_, _, _, flen) in subcols)
    n_mm_tp = sum(len(range(0, flen, P)) for (_, _, _, flen) in subcols)

    with ExitStack() as ctx:
        tc = ctx.enter_context(tile.TileContext(nc))
        dma_pool = ctx.enter_context(tc.tile_pool(name="dma", bufs=2))
        dpool = ctx.enter_context(tc.tile_pool(name="d", bufs=2))
        prpool = ctx.enter_context(tc.tile_pool(name="pred", bufs=2))
        gpool = ctx.enter_context(tc.tile_pool(name="garbage", bufs=1))
        apool = ctx.enter_context(tc.tile_pool(name="accs", bufs=1))
        pspool = ctx.enter_context(tc.tile_pool(name="ps", bufs=1, space="PSUM"))

        spf_a = apool.tile([P, n_tiles], f32, tag="spf_a")
        ld_a = apool.tile([P, ncol], f32, tag="ld_a")
        ones = apool.tile([P, 1], bf16, tag="ones")
        nc.vector.memset(ones, 1.0)
        ps_p1 = pspool.tile([1, w_p1], f32, tag="ps_p1")
        ps_tp = pspool.tile([P, P], f32, tag="ps_tp")

        g_e = gpool.tile([P, max(plan)], f32, tag="g_e")
        g_ld = gpool.tile([P, f_c], f32, tag="g_ld")

        mm1 = 0
        mm2 = 0
        row = 0
        for ti, f in enumerate(plan):
            ot = dma_pool.tile([P, 2 * f], f32, tag="ot")
            lt = dma_pool.tile([P, f], bf16, tag="lt")
            nc.sync.dma_start(
                out=ot, in_=outs[row:row + P * f].rearrange("(p f) c -> p (f c)", p=P))
            nc.sync.dma_start(
                out=lt, in_=labs[row:row + P * f].rearrange("(p f) -> p f", p=P))
            row += P * f
            ot3 = ot.rearrange("p (f c) -> p f c", c=2)
            x0 = ot3[:, :, 0]
            x1 = ot3[:, :, 1]
            dt_ = dpool.tile([P, f], f32, tag="d")
            if d_on_gpsimd:
                nc.gpsimd.tensor_tensor(out=dt_, in0=x1, in1=x0, op=Alu.subtract)
            else:
                nc.vector.tensor_tensor(out=dt_, in0=x1, in1=x0, op=Alu.subtract)
            # softplus(d) = ln(exp(d) + 1); Exp and Ln share the
            # natural_log_exp_and_others ACT table set (single load).
            # |d| <~ 9 here so exp(d) stays well inside fp32 range.
            nc.scalar.activation(out=g_e[:, :f], in_=dt_, func=Act.Exp)
            nc.scalar.activation(
                out=g_e[:, :f], in_=g_e[:, :f], func=Act.Ln, bias=1.0,
                accum_out=spf_a[:, ti:ti + 1],
            )
            col0 = sum(1 for (tj, _, _, _) in subcols if tj < ti)
            for si, (tj, _, off, flen) in enumerate(
                    (s for s in subcols if s[0] == ti)):
                col = col0 + si
                sl = slice(off, off + flen)
                nc.vector.scalar_tensor_tensor(
                    out=g_ld[:, :flen], in0=dt_[:, sl], scalar=0.0,
                    in1=lt[:, sl], op0=Alu.bypass, op1=Alu.mult,
                    accum_out=ld_a[:, col:col + 1],
                )
                pred = prpool.tile([P, flen], bf16, tag="pred")
                nc.vector.tensor_scalar(
                    out=pred, in0=dt_[:, sl], scalar1=0.0, scalar2=None,
                    op0=Alu.is_gt,
                )
                for m in range(0, flen, w_p1):
                    nc.tensor.matmul(
                        ps_p1[:, :], lhsT=ones, rhs=pred[:, m:m + w_p1],
                        start=(mm1 == 0), stop=(mm1 == n_mm_p1 - 1))
                    mm1 += 1
                for c in range(0, flen, P):
                    nc.tensor.matmul(
                        ps_tp[:, :], lhsT=lt[:, off + c:off + c + P],
                        rhs=pred[:, c:c + P],
                        start=(mm2 == 0), stop=(mm2 == n_mm_tp - 1))
                    mm2 += 1

        p1_sb = apool.tile([1, w_p1], f32, tag="p1_sb")
        tp_sb = apool.tile([P, P], f32, tag="tp_sb")
        nc.vector.tensor_copy(out=p1_sb, in_=ps_p1)
        nc.vector.tensor_copy(out=tp_sb, in_=ps_tp)
        nc.sync.dma_start(out=spf_o[:, :], in_=spf_a)
        nc.sync.dma_start(out=ld_o[:, :], in_=ld_a)
        nc.sync.dma_start(out=p1_o[:, :], in_=p1_sb)
        nc.sync.dma_start(out=tp_o[:, :], in_=tp_sb)

    nc.finalize()
    return nc, ncol


def make_in_maps(outputs, labels):
    """Shard full inputs into per-core in_maps (labels cast to f32)."""
    import ml_dtypes

    outputs = np.asarray(outputs)
    if outputs.dtype != np.float32:
        outputs = outputs.astype(np.float32)
    # bf16 is exact for labels in {0,1} and halves the label DMA bytes
    labels_f = np.asarray(labels).astype(ml_dtypes.bfloat16)
    in_maps = []
    for c in range(N_CORES):
        sl = slice(c * N_LOC, (c + 1) * N_LOC)
        in_maps.append({"outputs": outputs[sl], "labels": labels_f[sl]})
    return in_maps


def finish_host(per_core_results, n1, n_total=N_TOTAL):
    """Combine per-core partial sums into the final scalar (float64 math).

    n1 = exact sum(labels), computed host-side. p1 comes from the PE
    ones-matmul; TP is the trace of the PE chunk-product accumulator.
    """
    s_spf = s_ld = tp = p1 = 0.0
    for r in per_core_results:
        s_spf += float(np.sum(r["spf_p"], dtype=np.float64))
        s_ld += float(np.sum(r["ld_p"], dtype=np.float64))
        tp += float(np.trace(r["tp_p"].astype(np.float64)))
        p1 += float(np.sum(r["p1_p"], dtype=np.float64))

    n1 = float(n1)
    ce_mean = (s_spf - s_ld) / n_total
    fn = n1 - tp
    fp = p1 - tp
    tn = n_total - n1 - p1 + tp
    all_nonzero = (tp != 0.0) and (tn != 0.0) and (fp != 0.0) and (fn != 0.0)
    sens = tp / max(tp + fn, 1.0)
    prec = tp / max(tp + fp, 1.0)
    gm_log = -0.5 * np.log(max(sens * prec, 1e-30))
    coeff = gm_log * LAMBD if all_nonzero else LAMBD
    cs_mean = fn / n_total
    return np.asarray(ce_mean + coeff * cs_mean, dtype=np.float32)


_CACHED = {}


def kernel(outputs, labels):
    from concourse.bass_utils import run_bass_kernel_spmd

    if "nc" not in _CACHED:
        _CACHED["nc"], _ = build_bass_kernel()
    nc = _CACHED["nc"]
    n1 = int(np.asarray(labels).sum())  # exact (labels are 0/1 ints)
    in_maps = make_in_maps(outputs, labels)
    res = run_bass_kernel_spmd(nc, in_maps, core_ids=list(range(N_CORES)))
    return finish_host(res.results, n1)
